# revision 47
# baseline (speedup 1.0000x reference)
"""Causal dense attention (key=value) on 8 TRN2 NeuronCores.

Reference semantics (B=4, T=2048, D=1024, fp32):
    scores  = Q @ V^T                      [B, T, T]
    scores -= 1e9 * (~tril)                causal mask
    W       = softmax(scores, axis=-1)
    out     = W @ V                        [B, T, D]

Sharding: 2 cores per batch. Each batch's 16 causal q-tiles (128 rows
each, kv extent 128*(t+1)) are split odd/even so both cores get the
same padded kv-extent schedule EXT = [256, 512, ..., 2048], making the
Bass program identical across all 8 cores (pure SPMD). Padding columns
are killed by the additive causal mask, which is the same [128, 256]
pattern for every slot of a given core.

Host stages per core: Q^T in a slot-blocked layout (qt[j, dp, d8, c],
2KB contiguous per partition so single-slot DMAs run at full DMA rate),
V^T (d-major) in fp16, V split into fp8e4 hi/lo halves (V = hi + lo at
~bf16 effective precision), and one additive causal mask tile.

Device schedule: mm1 is emitted chunk-interleaved rather than
slot-major: while the input stream lands (the DMA wire is saturated for
the first ~31us), the PE walks (slot, kv-chunk) score strips in slot
order 0,1,3,4,5,7,2,6, matching the wire's delivery order so it never
waits long for the next chunk; input DMA pieces are >= 256KB so the
one-dispatch-per-625ns HWDGE cadence never starves the wire. Each
strip's row-max (and the final strip's causal-mask add) is emitted
right after its matmuls, so nmax is ready shortly after a slot's last
strip; stats(j) then only runs the exps (ScalarE, fused bias, fp8 W
out) and row-sums of the quantized W (DVE) so the softmax normalizer
matches the fp8 weights exactly. Score strips live in a 4-deep PSUM
pool from mm1 until their slot's exp; the emission order places back()
work between a slot's stats and the strip allocations that reuse its
banks, so the exp latency is always shadowed by mm2 work. Backs are
split into tr(j) (PE transposes in 8-block groups + repack on ACT/DVE,
emitted early so repacks sit ahead of later exps in queue order) and
back(j) (fp8 DoubleRow mm2 into two 512-wide PSUM chunks, scale-fused
out-copies, DMA out). The tail is choreographed: back(7) stores from
DVE+SP to keep ACT clear, back(6) runs chunk-major with tr(2)
sandwiched between its chunks so repack(2) overlaps chunk-1 matmuls,
and the last back (slot 2) runs chunk-major with chunk 0's chain on
ACT under chunk 1's matmuls and chunk 1 copying on DVE and shipping
from the idle SP queue.
"""

import numpy as np

import concourse.bass as bass
import concourse.mybir as mybir
from concourse import bacc, tile
from concourse.bass_utils import run_bass_kernel_spmd
from concourse.masks import make_identity

import ml_dtypes

B, T, D = 4, 2048, 1024
NCORES = 8
NSLOT = 8
EXT = [256 * (j + 1) for j in range(NSLOT)]  # kv extent per slot
NEG_INF = 1e9
PE_WARMUP = 36  # dependency-free PE transposes at program start

F32 = mybir.dt.float32
F16 = mybir.dt.float16
BF16 = mybir.dt.bfloat16
FP8 = mybir.dt.float8e4
FP8E5 = mybir.dt.float8e5
E4 = ml_dtypes.float8_e4m3


def _tiles_for_core(c):
    """q-tile index (within the batch) for each slot, for core c."""
    if c < 4:
        return [2 * j + 1 for j in range(NSLOT)]  # extents exactly EXT
    return [2 * j for j in range(NSLOT)]  # extents EXT - 128 (padded)


def _build_program():
    nc = bacc.Bacc("TRN2", target_bir_lowering=False)

    # qt[j, dp, d8, c] = Q^T[d8*128+dp, col c of slot j]; per-partition
    # runs are 2KB so single-slot DMAs hit full DMA rate
    qt_d = nc.dram_tensor("qt", [NSLOT, 128, 8, 128], F16, kind="ExternalInput")
    vt_d = nc.dram_tensor("vt", [D, T], F16, kind="ExternalInput")
    vhl_d = nc.dram_tensor("vhl", [2, T, D], FP8, kind="ExternalInput")
    mask_d = nc.dram_tensor("mask", [128, 256], FP8E5, kind="ExternalInput")
    o_d = nc.dram_tensor("o", [NSLOT * 128, D], BF16, kind="ExternalOutput")

    with tile.TileContext(nc) as tc:
        with (
            tc.tile_pool(name="const", bufs=1) as constp,
            tc.tile_pool(name="qt", bufs=1) as qtp,
            tc.tile_pool(name="vt", bufs=1) as vtp,
            tc.tile_pool(name="vn", bufs=1) as vnp,
            tc.tile_pool(name="w", bufs=1) as wp,
            tc.tile_pool(name="wt", bufs=1) as wtp,
            tc.tile_pool(name="osb", bufs=4) as op,
            tc.tile_pool(name="stats", bufs=24) as statp,
            tc.tile_pool(name="ps_s", bufs=4, space="PSUM") as ps_s,
            tc.tile_pool(name="ps_t", bufs=2, space="PSUM") as ps_t,
            tc.tile_pool(name="ps_o", bufs=2, space="PSUM") as ps_o,
        ):
            # PE p-state warm-up on a zero tile: does not wait for the
            # identity build, so the ramp starts at ~150ns
            wz = constp.tile([128, 128], FP8, tag="wz")
            nc.gpsimd.memset(wz[:], 0.0)
            wu_ps = ps_t.tile([128, 8, 128, 2], FP8, tag="tp")
            for _ in range(PE_WARMUP):
                nc.tensor.transpose(wu_ps[:, 0, :, 0], wz[:], wz[:])

            ident = constp.tile([128, 128], FP8, tag="ident")
            make_identity(nc, ident[:])

            # ACT exp-table warm-up: load exp_and_others during initial DMAs
            warm = statp.tile([128, 1], F32, tag="warm")
            nc.gpsimd.memset(warm[:], 0.0)
            nc.scalar.activation(warm[:], warm[:], mybir.ActivationFunctionType.Exp)

            masks = constp.tile([128, 256], FP8E5, tag="masks")

            qts = {}   # (d8, j) -> [128, 128] AP
            vts = {}   # (d8, kc) -> [128, 512]
            vt_tiles = {}  # kc -> chunk tile
            vh_pair = {}  # g -> [128, 2, D] AP (rows g*256 + i*128 + p)
            vl_pair = {}

            qt_tiles = {}

            def dma_qt(j, lo=True, hi=True):
                if j not in qt_tiles:
                    t_ = qtp.tile([128, 8, 128], F16, tag=f"qt{j}")
                    qt_tiles[j] = t_
                    for d8 in range(8):
                        qts[(d8, j)] = t_[:, d8, :]
                t_ = qt_tiles[j]
                if lo and hi:
                    nc.sync.dma_start(t_[:], qt_d[j])
                elif lo:
                    nc.sync.dma_start(t_[:, 0:4, :], qt_d[j, :, 0:4, :])
                elif hi:
                    nc.sync.dma_start(t_[:, 4:8, :], qt_d[j, :, 4:8, :])

            def dma_vt(kc, h0, hw, lo=True, hi=True):
                """Load cols [kc*512+h0, +hw) of V^T into chunk tile kc."""
                if kc not in vt_tiles:
                    t_ = vtp.tile([128, 8, 512], F16, tag=f"vtw{kc}")
                    vt_tiles[kc] = t_
                    for d8 in range(8):
                        vts[(d8, kc)] = t_[:, d8, :]
                t_ = vt_tiles[kc]
                a0 = kc * 512 + h0
                src = vt_d[:, a0 : a0 + hw].rearrange("(a p) k -> p a k", p=128)
                if lo and hi:
                    nc.sync.dma_start(t_[:, :, h0 : h0 + hw], src)
                elif lo:
                    nc.sync.dma_start(t_[:, 0:4, h0 : h0 + hw], src[:, 0:4, :])
                elif hi:
                    nc.sync.dma_start(t_[:, 4:8, h0 : h0 + hw], src[:, 4:8, :])

            def dma_vhl(q_):
                # one wave carries BOTH the fp8 hi and lo halves of 512 rows
                t_ = vnp.tile([128, 2, 4, D], FP8, tag=f"vw{q_}")
                for ti in range(2):
                    nc.sync.dma_start(
                        t_[:, ti],
                        vhl_d[ti, q_ * 512 : (q_ + 1) * 512, :].rearrange(
                            "(a p) d -> p a d", p=128
                        ),
                    )
                for li, pair in ((0, vh_pair), (1, vl_pair)):
                    pair[q_ * 2] = t_[:, li, 0:2, :]
                    pair[q_ * 2 + 1] = t_[:, li, 2:4, :]

            strips = {}  # j -> list of (tile, c0, w)
            nmaxs = {}   # j -> running negated row max
            w_tiles = {}
            rinvs = {}

            def mm1(j, c0, w, final=False):
                """One score strip: cols [c0, c0+w) of slot j. The strip's
                row-max (and, for the slot's final strip, the causal mask
                add) is emitted immediately so nmax is ready shortly after
                the last strip's matmuls."""
                s_ = ps_s.tile([128, 512], F32, tag="sw")
                kc = c0 // 512
                o0 = c0 % 512
                for d8 in range(8):
                    nc.tensor.matmul(
                        s_[:, 0:w],
                        qts[(d8, j)],
                        vts[(d8, kc)][:, o0 : o0 + w],
                        start=(d8 == 0),
                        stop=(d8 == 7),
                    )
                strips.setdefault(j, []).append((s_, c0, w))
                if final:
                    # additive causal mask on the last 256 columns (DVE)
                    nc.vector.tensor_tensor(
                        s_[:, w - 256 : w],
                        s_[:, w - 256 : w],
                        masks[:],
                        op=mybir.AluOpType.add,
                    )
                nm = statp.tile([128, 1], F32, tag="nm")
                nc.vector.reduce_max(
                    nm[:], s_[:, :w], axis=mybir.AxisListType.X, negate=True
                )
                if j in nmaxs:
                    nm2 = statp.tile([128, 1], F32, tag="nmc")
                    nc.vector.tensor_tensor(
                        nm2[:], nmaxs[j][:], nm[:], op=mybir.AluOpType.min
                    )
                    nmaxs[j] = nm2
                else:
                    nmaxs[j] = nm

            def stats(j):
                """exp + row-sums for slot j; frees its strips."""
                E = EXT[j]
                wins = strips[j]
                nmax = nmaxs[j]
                # exp (fused bias) -> fp8 W in SBUF; row-sum of the QUANTIZED
                # weights via DVE so normalization cancels fp8 rounding
                w_sb = wp.tile([128, E], FP8, tag=f"w{j}")
                rsum = None
                for s_, c0, w in wins:
                    nc.scalar.activation(
                        w_sb[:, c0 : c0 + w],
                        s_[:, :w],
                        mybir.ActivationFunctionType.Exp,
                        bias=nmax[:],
                    )
                    rs = statp.tile([128, 1], F32, tag="rs")
                    nc.vector.tensor_reduce(
                        rs[:], w_sb[:, c0 : c0 + w],
                        axis=mybir.AxisListType.X, op=mybir.AluOpType.add,
                    )
                    if rsum is None:
                        rsum = rs
                    else:
                        rs2 = statp.tile([128, 1], F32, tag="rsc")
                        nc.vector.tensor_add(rs2[:], rsum[:], rs[:])
                        rsum = rs2
                rinv = statp.tile([128, 1], F32, tag="rinv")
                nc.vector.reciprocal(rinv[:], rsum[:])
                w_tiles[j] = w_sb
                rinvs[j] = rinv

            wt_lists = {}

            def tr(j, g0_eng="act"):
                """transpose W (PE) + repack to SBUF (ACT/DVE)."""
                w_sb = w_tiles[j]
                nblk = EXT[j] // 128
                # transposes in 8-block groups; repack group 0 on ACT (or
                # DVE), the rest on DVE. Emitted well before mm2(j) so the
                # repacks sit ahead of later exps in their queues.
                wt_list = []
                for gi, g0 in enumerate(range(0, nblk, 8)):
                    gn = min(8, nblk - g0)
                    t_ps = ps_t.tile([128, 8, 128, 2], FP8, tag="tp")
                    for bi in range(gn):
                        blk = g0 + bi
                        nc.tensor.transpose(
                            t_ps[:, bi, :, 0],
                            w_sb[:, blk * 128 : (blk + 1) * 128],
                            ident[:],
                        )
                    wt_sb = wtp.tile([128, 1024], FP8, tag=f"wt{j}_{gi}")
                    if g0_eng == "split":
                        # latency-critical repack: 2-block pieces alternating
                        # ACT/DVE, landing in the order mm2 consumes pairs
                        for p0 in range(0, gn, 2):
                            pn = min(2, gn - p0)
                            if (p0 // 2) % 2 == 0:
                                nc.scalar.activation(
                                    wt_sb[:, p0 * 128 : (p0 + pn) * 128],
                                    t_ps[:, p0 : p0 + pn, :, 0],
                                    mybir.ActivationFunctionType.Copy,
                                )
                            else:
                                nc.vector.tensor_copy(
                                    wt_sb[:, p0 * 128 : (p0 + pn) * 128],
                                    t_ps[:, p0 : p0 + pn, :, 0],
                                )
                    elif gi == 0 and g0_eng == "act":
                        nc.scalar.activation(
                            wt_sb[:, : gn * 128],
                            t_ps[:, :gn, :, 0],
                            mybir.ActivationFunctionType.Copy,
                        )
                    else:
                        nc.vector.tensor_copy(
                            wt_sb[:, : gn * 128], t_ps[:, :gn, :, 0]
                        )
                    wt_list.append(wt_sb)
                wt_lists[j] = wt_list

            def _mm2_chunk(j, o_ps, dd, dw):
                """Accumulate output cols [dd, dd+dw) of slot j into o_ps."""
                npair = EXT[j] // 256
                wt_list = wt_lists[j]
                for g in range(npair):
                    wt_ap = wt_list[g // 4][
                        :, (g % 4) * 256 : (g % 4 + 1) * 256
                    ].rearrange("p (i m) -> p i m", i=2)
                    for half, vpair in ((0, vh_pair), (1, vl_pair)):
                        nc.tensor.matmul(
                            o_ps[:, 0:dw],
                            wt_ap,
                            vpair[g][:, :, dd : dd + dw],
                            start=(half == 0 and g == 0),
                            stop=(half == 1 and g == npair - 1),
                            perf_mode=mybir.MatmulPerfMode.DoubleRow,
                        )

            o_sb_pre = {}

            def back(j, last=False, declog=False, mid=None, store_dve=False):
                """mm2 + store for slot j (tr(j) must have been emitted).

                normal: both 512-wide chunks accumulate simultaneously in two
                PSUM banks (hi/lo interleaved per pair, matching the arrival
                order of the V hi/lo waves); stores follow, overlapped by the
                next back's transposes/repacks.
                declog/last: chunk-major so chunk 0's chain runs under chunk
                1's matmuls; `mid` is emitted between the chunks (used to
                sandwich the next back's transposes); the final chunk copies
                on DVE and ships from the idle SP queue.
                """
                rinv = rinvs[j]
                E = EXT[j]
                wt_list = wt_lists[j]
                npair = E // 256
                o_sb = o_sb_pre.get(j)
                if o_sb is None:
                    o_sb = op.tile([128, 1024], BF16, tag="o", name="o_sb")
                o_c0 = ps_o.tile([128, 512], F32, tag="oc")
                o_c1 = ps_o.tile([128, 512], F32, tag="oc")
                if declog or last:
                    _mm2_chunk(j, o_c0, 0, 512)
                    if mid is not None:
                        # sandwiched transposes/repacks of the next back go
                        # ahead of this chunk's copy in the queues
                        mid()
                    nc.scalar.activation(
                        o_sb[:, 0:512],
                        o_c0[:],
                        mybir.ActivationFunctionType.Copy,
                        scale=rinv[:],
                    )
                    nc.scalar.dma_start(
                        o_d[j * 128 : (j + 1) * 128, 0:512], o_sb[:, 0:512]
                    )
                    _mm2_chunk(j, o_c1, 512, 512)
                    nc.vector.scalar_tensor_tensor(
                        o_sb[:, 512:1024],
                        o_c1[:],
                        rinv[:],
                        wt_list[0][:, 0:512],
                        op0=mybir.AluOpType.mult,
                        op1=mybir.AluOpType.bypass,
                    )
                    nc.sync.dma_start(
                        o_d[j * 128 : (j + 1) * 128, 512:1024],
                        o_sb[:, 512:1024],
                    )
                    return
                o_chunks = [o_c0, o_c1]
                for g in range(npair):
                    wt_ap = wt_list[g // 4][
                        :, (g % 4) * 256 : (g % 4 + 1) * 256
                    ].rearrange("p (i m) -> p i m", i=2)
                    for half, vpair in ((0, vh_pair), (1, vl_pair)):
                        for ci, dd in enumerate((0, 512)):
                            nc.tensor.matmul(
                                o_chunks[ci][:],
                                wt_ap,
                                vpair[g][:, :, dd : dd + 512],
                                start=(half == 0 and g == 0),
                                stop=(half == 1 and g == npair - 1),
                                perf_mode=mybir.MatmulPerfMode.DoubleRow,
                            )
                if store_dve:
                    # keep the ACT queue free for the tail's exps/repacks:
                    # both copies on DVE, ship from SP
                    for ci, dd in enumerate((0, 512)):
                        nc.vector.scalar_tensor_tensor(
                            o_sb[:, dd : dd + 512],
                            o_chunks[ci][:],
                            rinv[:],
                            wt_list[0][:, 0:512],
                            op0=mybir.AluOpType.mult,
                            op1=mybir.AluOpType.bypass,
                        )
                    nc.sync.dma_start(o_d[j * 128 : (j + 1) * 128, :], o_sb[:])
                    return
                for ci, dd in enumerate((0, 512)):
                    nc.scalar.activation(
                        o_sb[:, dd : dd + 512],
                        o_chunks[ci][:],
                        mybir.ActivationFunctionType.Copy,
                        scale=rinv[:],
                    )
                nc.scalar.dma_start(o_d[j * 128 : (j + 1) * 128, :], o_sb[:])

            # ---- the schedule ----
            # DMA emissions (SP queue order = wire order) are interleaved
            # with PE emissions so each strip's data lands just before the
            # PE reaches it.
            # first pieces are >= 256KB so the wire is never starved by the
            # one-dispatch-per-625ns HWDGE cadence
            dma_qt(0)
            dma_vt(0, 0, 256)                 # c0a
            dma_qt(1)
            dma_vt(0, 256, 256)               # c0b
            nc.sync.dma_start(masks[:], mask_d[:, :])
            dma_qt(3)
            dma_qt(4)

            mm1(0, 0, 256, final=True)
            stats(0)
            mm1(1, 0, 256)
            mm1(1, 256, 256, final=True)
            stats(1)

            dma_vt(1, 0, 512)                 # c1
            dma_vt(2, 0, 256)                 # c2a
            dma_vhl(0)                        # rows 0-512

            mm1(3, 0, 512)
            tr(0, g0_eng="act")
            mm1(4, 0, 512)
            tr(1, g0_eng="act")
            mm1(3, 512, 512, final=True)
            stats(3)
            mm1(4, 512, 512)
            mm1(4, 1024, 256, final=True)
            stats(4)

            dma_qt(5)
            dma_vt(2, 256, 256)               # c2b

            back(0)
            back(1)

            mm1(5, 0, 512)
            tr(3, g0_eng="dve")
            mm1(5, 512, 512)

            dma_qt(7)
            dma_vhl(1)                        # rows 512-1024

            mm1(5, 1024, 512, final=True)
            stats(5)
            mm1(7, 0, 512)

            dma_vt(3, 0, 512)                 # c3

            back(3)

            mm1(7, 512, 512)
            tr(4, g0_eng="dve")

            dma_qt(2)
            dma_qt(6)

            mm1(7, 1024, 512)
            tr(5, g0_eng="dve")
            mm1(7, 1536, 512, final=True)
            stats(7)

            dma_vhl(2)                        # rows 1024-1536

            back(4)

            dma_vhl(3)                        # rows 1536-2048

            back(5)

            mm1(6, 0, 512)
            tr(7, g0_eng="act")
            mm1(6, 512, 512)
            mm1(6, 1024, 512)
            mm1(6, 1536, 256, final=True)
            stats(6)

            back(7, store_dve=True)

            mm1(2, 0, 512)
            tr(6, g0_eng="act")
            mm1(2, 512, 256, final=True)
            stats(2)

            back(6, declog=True, mid=lambda: tr(2, g0_eng="act"))
            back(2, last=True)

    nc.finalize()
    return nc


_NC_CACHE = None


def _get_program():
    global _NC_CACHE
    if _NC_CACHE is None:
        _NC_CACHE = _build_program()
    return _NC_CACHE


def stage_inputs(query, value):
    """Build the 8 per-core input maps from the full query/value arrays."""
    query = np.asarray(query, dtype=np.float32)
    value = np.asarray(value, dtype=np.float32)

    vhl_b = []
    vt_b = []
    for b in range(B):
        vh = value[b].astype(E4)
        vl = (value[b] - vh.astype(np.float32)).astype(E4)
        vhl_b.append(np.ascontiguousarray(np.stack([vh, vl])))  # [2, T, D]
        vt_b.append(np.ascontiguousarray(value[b].T).astype(np.float16))

    in_maps = []
    for c in range(NCORES):
        b = c % 4
        tiles = _tiles_for_core(c)

        # qt[j, dp, d8, c] = query[b, tile_j*128 + c, d8*128 + dp]
        qt = np.empty((NSLOT, 128, 8, 128), dtype=np.float16)
        for j in range(NSLOT):
            qq = query[b, tiles[j] * 128 : (tiles[j] + 1) * 128, :]  # [c, d]
            qt[j] = qq.T.reshape(8, 128, 128).transpose(1, 0, 2)

        # same additive mask pattern for every slot of this core:
        # odd tiles (c<4): kill cols k where k > 128 + r of the last 256;
        # even tiles (c>=4): kill k > r (incl. the fully-padded last 128)
        r = np.arange(128)[:, None]
        k = np.arange(256)[None, :]
        mask = np.where(k > ((128 + r) if c < 4 else r), -57344.0, 0.0).astype(
            ml_dtypes.float8_e5m2
        )

        in_maps.append(
            {"qt": qt, "vt": vt_b[b], "vhl": vhl_b[b], "mask": mask}
        )
    return in_maps


def kernel(query, value):
    nc = _get_program()
    in_maps = stage_inputs(query, value)
    res = run_bass_kernel_spmd(nc, in_maps, core_ids=list(range(NCORES)))

    out = np.empty((B, T, D), dtype=np.float32)
    for c in range(NCORES):
        o = np.asarray(res.results[c]["o"]).astype(np.float32)  # [1024, D]
        b = c % 4
        for j, t in enumerate(_tiles_for_core(c)):
            out[b, t * 128 : (t + 1) * 128, :] = o[j * 128 : (j + 1) * 128, :]
    return out
